# revision 3
# baseline (speedup 1.0000x reference)
"""Diagonally-masked multi-head self-attention on 8 TRN2 NeuronCores.

Sharding (per the tensor/data-parallel hint, hardcoded):
  core c in 0..7 -> batch b = c // 4, head group g = c % 4 (4 heads each).
  Each core computes its batch's attention for its 4 heads plus the partial
  output projection (rows of Wo for its heads); the 4 partial outputs per
  batch are summed on the host (the "all-reduce").

bf16 pipeline, tuned against HW slope measurements (rel err 4.7e-3;
HW steady state ~183-196us/rep vs the 211.6us baseline):
  - all matmul operands (x, W*, QT/KT, ET, V, otn) are bf16; accumulation
    stays fp32 in PSUM, softmax denominators/reciprocals stay fp32.
  - host pre-lays-out every DRAM tensor in its SBUF tile order (partition
    dim first), so every DMA is a straight contiguous copy.
  - V projection lands in vaug padded to 128 cols (64 V dims + ones col +
    zeros): M=128 stationary enables fast weight load on the PV matmuls.
  - diagonal masks run on the otherwise-idle GPSIMD engine, keeping the
    strict-FIFO DVE queue clear for PSUM->SBUF copies (exp is the
    bottleneck engine; DVE stalls cascade into it).
  - xT for rep r+1 prefetches during rep r (big pool holds 2 reps) and
    rep r's projection tail pre-computes rep r+1's kt0/qt0 groups, so the
    exp stream restarts immediately at the rep seam.
  - output tiles ship as bf16 and are summed on the host in fp32.
"""

import numpy as np
import ml_dtypes

import concourse.bass as bass
import concourse.mybir as mybir
import concourse.tile as tile
from concourse import bacc
from concourse.bass_utils import run_bass_kernel_spmd

B, L, DIM = 2, 2048, 1024
H, D = 16, 64
NCORES = 8
HPC = 4  # heads per core
GCOLS = HPC * D  # 256 weight cols per core
KCH = DIM // 128  # 8 contraction chunks for the projections
QC = L // 512  # 4 query chunks
JT = L // 128  # 16 key tiles
SCALE = 1.0 / 8.0  # 1/sqrt(D)

F32 = mybir.dt.float32
F32R = mybir.dt.float32r
BF16 = mybir.dt.bfloat16
NPBF = ml_dtypes.bfloat16
EXP = mybir.ActivationFunctionType.Exp


_NC_CACHE = {}


def _build_nc(reps=1, loop=None):
    key = (reps, loop)
    if key in _NC_CACHE:
        return _NC_CACHE[key]

    nc = bacc.Bacc("TRN2", target_bir_lowering=False, debug=False, num_devices=NCORES)

    xT_d = nc.dram_tensor("xT", [128, KCH, L], BF16, kind="ExternalInput")
    # pair-major layout: [pair, 128, KCH, 128] so each pair's half is one
    # contiguous DMA (the pair-0 half gates the first QK groups)
    wq_d = nc.dram_tensor("wq", [2, 128, KCH, 128], BF16, kind="ExternalInput")
    wk_d = nc.dram_tensor("wk", [2, 128, KCH, 128], BF16, kind="ExternalInput")
    wv_d = nc.dram_tensor("wv", [128, KCH, HPC, D + 1], BF16, kind="ExternalInput")
    wo_d = nc.dram_tensor("wo", [128, 2, DIM], BF16, kind="ExternalInput")
    out_d = nc.dram_tensor("out", [L, DIM], BF16, kind="ExternalOutput")
    diag_d = nc.inline_tensor(
        np.ascontiguousarray((1.0 - np.eye(128)).astype(NPBF)), name="diagmask"
    )

    with tile.TileContext(nc) as tc:
        with (
            tc.tile_pool(name="singles", bufs=1) as singles,
            tc.tile_pool(name="big", bufs=16) as big,
            tc.tile_pool(name="etp", bufs=12) as etp,
            tc.tile_pool(name="otn", bufs=6) as otnp,
            tc.tile_pool(name="tmpp", bufs=3) as tmpp,
            tc.tile_pool(name="osb", bufs=4) as outp,
            tc.tile_pool(name="rd", bufs=4) as rdp,
            tc.tile_pool(name="bp", bufs=2, space="PSUM") as bp,
            tc.tile_pool(name="otps", bufs=2, space="PSUM") as otp,
            tc.tile_pool(name="smp", bufs=2, space="PSUM") as smp,
        ):
            # ---- static loads -------------------------------------------
            wq_t = singles.tile([128, 2, KCH, 128], BF16, tag="wq")
            wk_t = singles.tile([128, 2, KCH, 128], BF16, tag="wk")
            wv_t = singles.tile([128, KCH, HPC, D + 1], BF16, tag="wv")
            wo_t = singles.tile([128, 2, DIM], BF16, tag="wo")
            # head 3's Wo rows also staged at base partition 0: the last
            # chunk's projection reads its unshifted bottom half via split-K
            wo_bt = singles.tile([D, DIM], BF16, tag="wobt")
            diag_t = singles.tile([128, 128], BF16, tag="diag")
            ones_t = singles.tile([128, 64], F32R, tag="ones")
            # PV stationary padded to 128 cols: rows 0:D = V dims, row D =
            # denominator ones, rows D+1:127 = zeros. M=128 enables FWL.
            vaug = singles.tile([128, JT, HPC, 128], BF16, tag="vaug")
            qt = [singles.tile([128, L], BF16, tag=f"qt{p}", name=f"qt{p}") for p in range(2)]
            kt = [singles.tile([128, L], BF16, tag=f"kt{p}", name=f"kt{p}") for p in range(2)]

            # pair-0 halves of Wk/Wq ship first: they gate the first QK groups
            nc.sync.dma_start(out=wk_t[:, 0], in_=wk_d[0])
            nc.sync.dma_start(out=wq_t[:, 0], in_=wq_d[0])
            nc.vector.memset(ones_t[:].bitcast(F32), 1.0)
            nc.vector.memset(vaug[:].bitcast(mybir.dt.uint16), 0)
            nc.vector.memset(vaug[:, :, :, D].bitcast(mybir.dt.uint16), 0x3F80)

            def load_late_weights():
                nc.sync.dma_start(out=diag_t, in_=diag_d[:])
                nc.sync.dma_start(out=wv_t, in_=wv_d[:])
                nc.sync.dma_start(out=wk_t[:, 1], in_=wk_d[1])
                nc.sync.dma_start(out=wq_t[:, 1], in_=wq_d[1])
                nc.sync.dma_start(out=wo_t, in_=wo_d[:])
                nc.sync.dma_start(out=wo_bt, in_=wo_d[D:128, 1, :])

            def ship_xt(first):
                    # xT ships in c4-major quarter-chunks: the preamble's
                    # kt0/qt0 groups only touch columns 0-511 of each chunk,
                    # so the first scores can fire ~6us in instead of ~15us
                    xt = [
                        big.tile([128, L], BF16, tag="big", name=f"xt{k}")
                        for k in range(KCH)
                    ]
                    for c4 in range(QC):
                        for k in range(KCH):
                            nc.sync.dma_start(
                                out=xt[k][:, 512 * c4 : 512 * (c4 + 1)],
                                in_=xT_d[:, k, 512 * c4 : 512 * (c4 + 1)],
                            )
                        if first and c4 == 0:
                            # diag gates the j=0 mask, wv the slot-0 V group
                            nc.sync.dma_start(out=diag_t, in_=diag_d[:])
                            nc.sync.dma_start(out=wv_t, in_=wv_d[:])
                    return xt

            def late_weights_first():
                    nc.sync.dma_start(out=wk_t[:, 1], in_=wk_d[1])
                    nc.sync.dma_start(out=wq_t[:, 1], in_=wq_d[1])
                    nc.sync.dma_start(out=wo_t, in_=wo_d[:])
                    nc.sync.dma_start(out=wo_bt, in_=wo_d[D:128, 1, :])

            def body(first, xt=None, nxt=None, pre01_done=False):
                    if xt is None:
                        xt = ship_xt(first)
                        if first:
                            late_weights_first()
                    def qk_mm(ps, pair, qk, c4, k, xtl=None):
                        wt = (wq_t, wk_t)[qk]
                        xl = xt if xtl is None else xtl
                        nc.tensor.matmul(
                            out=ps,
                            lhsT=wt[:, pair, k, :],
                            rhs=xl[k][:, 512 * c4 : 512 * (c4 + 1)],
                            start=(k == 0),
                            stop=(k == KCH - 1),
                        )

                    def qk_finish(ps, pair, qk, c4, eng="v"):
                        dst = (qt, kt)[qk][pair][:, 512 * c4 : 512 * (c4 + 1)]
                        if eng == "s":
                            # boundary-critical copies ride the scalar engine,
                            # which idles there while DVE is backlogged
                            nc.scalar.copy(out=dst, in_=ps)
                        else:
                            nc.vector.tensor_copy(out=dst, in_=ps)

                    def qk_group(pair, qk, c4, eng="v"):
                        """One [128, 512] accumulation group of QT or KT."""
                        nm = f"ps{'qk'[qk]}{pair}_{c4}"
                        ps = smp.tile([128, 512], F32, tag="sm", name=nm)
                        for k in range(KCH):
                            qk_mm(ps, pair, qk, c4, k)
                        qk_finish(ps, pair, qk, c4, eng)

                    def v_group(t, pool=None, tag=None):
                        p, tg = (pool or smp), (tag or "sm")
                        ps = p.tile([128, HPC, D + 1], F32, tag=tg, name=f"psv{t}")
                        for k in range(KCH):
                            nc.tensor.matmul(
                                out=ps,
                                lhsT=xt[k][:, 128 * t : 128 * (t + 1)],
                                rhs=wv_t[:, k, :, :],
                                start=(k == 0),
                                stop=(k == KCH - 1),
                            )
                        nc.vector.tensor_copy(
                            out=vaug[:, t, :, 0:D], in_=ps[:, :, 0:D]
                        )

                    # ---- preamble: 4 QK accumulation groups consume each xT
                    # chunk as its DMA lands, so the first scores trail the last
                    # chunk by ~1 matmul instead of 2 serial groups + copies.
                    if first:
                        # p-state warmup: keep PE busy on throwaway matmuls
                        # through the initial DMA window so the real preamble
                        # runs at full clock (cold PE is 2-3.7x slower)
                        warm = bp.tile([128, 512], F32, tag="bp", name="warm")
                        for _ in range(16):
                            nc.tensor.matmul(
                                out=warm[0:64, 0:64],
                                lhsT=ones_t,
                                rhs=ones_t[:, 0:64],
                                start=True,
                                stop=True,
                            )
                    if not pre01_done:
                        # kt0/qt0 c4=0 groups (skipped when the previous rep's
                        # tail already computed them from the prefetched xT)
                        pre_kt0 = smp.tile([128, 512], F32, tag="sm", name="pre_kt0")
                        pre_qt0 = smp.tile([128, 512], F32, tag="sm", name="pre_qt0")
                        for k in range(KCH):
                            qk_mm(pre_kt0, 0, 1, 0, k)
                            qk_mm(pre_qt0, 0, 0, 0, k)
                        qk_finish(pre_kt0, 0, 1, 0)
                        qk_finish(pre_qt0, 0, 0, 0)
                    if not first:
                        # steady state: xT is prefetched, so kt1/kt2 can
                        # pre-start in the idle otp banks as before
                        pre_kt1 = otp.tile([128, 512], F32, tag="ot", name="pre_kt1")
                        pre_kt2 = otp.tile([128, 512], F32, tag="ot", name="pre_kt2")
                        for k in range(KCH):
                            qk_mm(pre_kt1, 0, 1, 1, k)
                            qk_mm(pre_kt2, 0, 1, 2, k)
                        qk_finish(pre_kt1, 0, 1, 1)
                        qk_finish(pre_kt2, 0, 1, 2)

                    otn = {}

                    def attn(c, pair, pre_av=None, extras=None, pv_delay=False):
                        # extras: list of (j_slot, thunk) placed inside the j loop
                        positions = dict(extras or [])
                        ha, hb = 2 * pair, 2 * pair + 1
                        ot_a = otp.tile([128, 512], F32, tag="ot", name=f"ota{c}_{pair}")
                        ot_b = otp.tile([128, 512], F32, tag="ot", name=f"otb{c}_{pair}")
                        otn[("ot", pair, c)] = (ot_a, ot_b)

                        def emit_pv(j, et):
                            # PV (+ denominator in row 64 via the ones column)
                            nc.tensor.matmul(
                                out=ot_a,
                                lhsT=vaug[:, j, ha, :],
                                rhs=et[:, 0:512],
                                start=(j == 0),
                                stop=(j == JT - 1),
                            )
                            nc.tensor.matmul(
                                out=ot_b,
                                lhsT=vaug[:, j, hb, :],
                                rhs=et[:, 512:1024],
                                start=(j == 0),
                                stop=(j == JT - 1),
                            )

                        pv_pend = None
                        for j in range(JT):
                            st = bp.tile([128, 1024], F32, tag="bp", name=f"st{c}_{pair}_{j}")
                            # scores (transposed): ST[k-tile, q-chunk]; the two heads
                            # of the pair run concurrently via row tiling.
                            nc.tensor.matmul(
                                out=st[:, 0:512],
                                lhsT=kt[pair][0:64, 128 * j : 128 * (j + 1)],
                                rhs=qt[pair][0:64, 512 * c : 512 * (c + 1)],
                                start=True,
                                stop=True,
                            )
                            nc.tensor.matmul(
                                out=st[:, 512:1024],
                                lhsT=kt[pair][64:128, 128 * j : 128 * (j + 1)],
                                rhs=qt[pair][64:128, 512 * c : 512 * (c + 1)],
                                start=True,
                                stop=True,
                            )
                            et = etp.tile([128, 1024], BF16, tag="et", name=f"et{c}_{pair}_{j}")
                            nc.scalar.activation(out=et, in_=st, func=EXP, scale=SCALE)
                            if 4 * c <= j < 4 * (c + 1):
                                off = 128 * (j - 4 * c)
                                # one strided TT masks both heads' diagonal
                                # blocks; diag broadcast via a 0-stride dim
                                ets = et[:, off : off + 128]
                                etv = bass.AP(
                                    ets.tensor, ets.offset,
                                    [ets.ap[0], [512, 2], [1, 128]],
                                )
                                dv = diag_t[:]
                                dbc = bass.AP(
                                    dv.tensor, dv.offset,
                                    [dv.ap[0], [0, 2], [1, 128]],
                                )
                                # mask rides the otherwise-idle GPSIMD engine so
                                # the exp->mask->PV chain never queues behind
                                # DVE copies (strict-FIFO DVE was a stall source)
                                nc.gpsimd.tensor_mul(out=etv, in0=etv, in1=dbc)
                            if pre_av is not None:
                                pre_av(j)
                            if j in positions:
                                positions[j]()
                            if pv_delay:
                                if pv_pend is not None:
                                    emit_pv(*pv_pend)
                                pv_pend = (j, et)
                            else:
                                emit_pv(j, et)
                        if pv_pend is not None:
                            emit_pv(*pv_pend)

                        # reciprocals queue on DVE right behind the last PV so
                        # the next attn's slot-0/1 PE broadcasts never stall
                        def emit_recip(h, ot):
                            rd = rdp.tile([D + 1, 512], F32R, tag="rd", name=f"rd{c}_{h}")
                            with nc.allow_low_precision(reason="1/D rounded to fp32r"):
                                nc.vector.reciprocal(
                                    out=rd[D : D + 1, :], in_=ot[D : D + 1, :]
                                )
                            return rd

                        rd_a = emit_recip(ha, ot_a)
                        rd_b = emit_recip(hb, ot_b)
                        otn[("rd", pair, c)] = (rd_a, rd_b)

                        def norm_half(h, ot, rd, top):
                            def run():
                                # broadcast 1/D (partition 64) to 64 partitions via PE
                                rdb_ps = smp.tile([D, 512], F32, tag="sm", name=f"rdps{c}_{h}")
                                nc.tensor.matmul(
                                    out=rdb_ps,
                                    lhsT=ones_t[D : D + 1, :],
                                    rhs=rd[D : D + 1, :],
                                    start=True,
                                    stop=True,
                                )
                                rdb = rdp.tile([D, 512], F32, tag="rdb", name=f"rdb{c}_{h}")
                                nc.vector.tensor_copy(out=rdb, in_=rdb_ps)
                                if top:
                                    # heads 0/2 land on partitions 0-63 of the paired tile
                                    otn2 = otnp.tile(
                                        [128, 512], BF16, tag="otn", name=f"otn{c}_{pair}"
                                    )
                                    otn[(pair, c)] = otn2
                                    nc.vector.tensor_mul(
                                        out=otn2[0:D, :], in0=ot[0:D, :], in1=rdb[:]
                                    )
                                else:
                                    # heads 1/3: normalize then DMA-shift to partitions 64-127
                                    tmp = tmpp.tile([D, 512], BF16, tag="tmp", name=f"otmp{c}_{pair}")
                                    nc.vector.tensor_mul(out=tmp, in0=ot[0:D, :], in1=rdb[:])
                                    nc.sync.dma_start(
                                        out=otn[(pair, c)][D : 2 * D, :], in_=tmp
                                    )

                            return run

                        return [
                            norm_half(ha, ot_a, rd_a, True),
                            norm_half(hb, ot_b, rd_b, False),
                        ]

                    def proj_group(c, tt, half):
                        t = 4 * c + tt
                        onp = smp.tile([128, 512], F32, tag="sm", name=f"onp{t}_{half}")
                        for g in range(2):
                            nc.tensor.matmul(
                                out=onp,
                                lhsT=otn[(g, c)][:, 128 * tt : 128 * (tt + 1)],
                                rhs=wo_t[:, g, 512 * half : 512 * (half + 1)],
                                start=(g == 0),
                                stop=(g == 1),
                            )
                        osb = outp.tile([128, 512], BF16, tag="osb", name=f"osb{t}_{half}")
                        nc.vector.tensor_copy(out=osb, in_=onp)
                        nc.sync.dma_start(
                            out=out_d[128 * t : 128 * (t + 1), 512 * half : 512 * (half + 1)],
                            in_=osb,
                        )

                    def proj_thunks(c):
                        return [
                            (lambda tt=tt, half=half: proj_group(c, tt, half))
                            for tt in range(4)
                            for half in range(2)
                        ]

                    def final_tail():
                        """Last chunk (pair 1) norms + projection, tuned for the
                        serial tail: interleaved norm chains (rdb copies on the
                        now-idle scalar engine), projection g=0 matmuls pre-issued
                        into six PSUM slots while the norms drain, output copies
                        split across scalar+vector, row-block-batched out DMAs."""
                        c = QC - 1
                        ot_a, ot_b = otn[("ot", 1, c)]
                        # reciprocals were already queued at the end of attn(3,1)
                        rd_a, rd_b = otn[("rd", 1, c)]
                        rps_a = smp.tile([D, 512], F32, tag="sm", name="f_rps_a")
                        nc.tensor.matmul(
                            out=rps_a, lhsT=ones_t[D : D + 1, :], rhs=rd_a[D : D + 1, :],
                            start=True, stop=True,
                        )
                        rps_b = smp.tile([D, 512], F32, tag="sm", name="f_rps_b")
                        nc.tensor.matmul(
                            out=rps_b, lhsT=ones_t[D : D + 1, :], rhs=rd_b[D : D + 1, :],
                            start=True, stop=True,
                        )
                        rdba = rdp.tile([D, 512], F32, tag="rdb", name="f_rdba")
                        nc.scalar.copy(out=rdba, in_=rps_a)
                        rdbb = rdp.tile([D, 512], F32, tag="rdb", name="f_rdbb")
                        nc.scalar.copy(out=rdbb, in_=rps_b)
                        otn2 = otnp.tile([128, 512], BF16, tag="otn", name="f_otn")
                        nc.vector.tensor_mul(out=otn2[0:D, :], in0=ot_a[0:D, :], in1=rdba)
                        bt = tmpp.tile([D, 512], BF16, tag="tmp", name="f_bt")
                        nc.vector.tensor_mul(out=bt, in0=ot_b[0:D, :], in1=rdbb)

                        onp = {}

                        def g0(q, pool, tag):
                            tt, half = q // 2, q % 2
                            o = pool.tile([128, 512], F32, tag=tag, name=f"f_onp{q}")
                            onp[q] = o
                            nc.tensor.matmul(
                                out=o,
                                lhsT=otn[(0, c)][:, 128 * tt : 128 * (tt + 1)],
                                rhs=wo_t[:, 0, 512 * half : 512 * (half + 1)],
                                start=True, stop=False,
                            )

                        osb = {}

                        def g1(q):
                            tt, half = q // 2, q % 2
                            o = onp[q]
                            nc.tensor.matmul(
                                out=o,
                                lhsT=otn2[0:D, 128 * tt : 128 * (tt + 1)],
                                rhs=wo_t[0:D, 1, 512 * half : 512 * (half + 1)],
                                start=False, stop=False,
                            )
                            nc.tensor.matmul(
                                out=o,
                                lhsT=bt[:, 128 * tt : 128 * (tt + 1)],
                                rhs=wo_bt[:, 512 * half : 512 * (half + 1)],
                                start=False, stop=True,
                            )
                            if tt not in osb:
                                osb[tt] = outp.tile(
                                    [128, 1024], BF16, tag="osb2", name=f"f_osb{tt}"
                                )
                            dst = osb[tt][:, 512 * half : 512 * (half + 1)]
                            if half == 0:
                                nc.scalar.copy(out=dst, in_=o)
                            else:
                                nc.vector.tensor_copy(out=dst, in_=o)
                            if half == 1:
                                t = 4 * c + tt
                                nc.sync.dma_start(
                                    out=out_d[128 * t : 128 * (t + 1), :], in_=osb[tt]
                                )

                        def nx_pre(qk, nm):
                            # next rep's kt0/qt0 c4=0 group from the prefetched
                            # xT, in a freed bp (st-pool) bank; overlaps the
                            # serial projection tail so the next rep's first
                            # exps fire immediately at the seam
                            ps = bp.tile([128, 512], F32, tag="bp", name=nm)
                            for k in range(KCH):
                                qk_mm(ps, 0, qk, 0, k, xtl=nxt)
                            qk_finish(ps, 0, qk, 0)

                        g0(0, smp, "sm")
                        g0(1, bp, "bp")
                        g0(2, bp, "bp")
                        g0(3, otp, "ot")
                        g0(4, otp, "ot")
                        g1(0)
                        g0(5, smp, "sm")
                        g1(1)
                        g0(6, smp, "sm")
                        g1(2)
                        g0(7, smp, "sm")
                        g1(3)
                        if nxt is not None:
                            nx_pre(1, "nx_kt0")
                        g1(4)
                        if nxt is not None:
                            nx_pre(0, "nx_qt0")
                        for q in range(5, 8):
                            g1(q)

                    def qk_thunk(pair, qk, c4, eng="v"):
                        return lambda: qk_group(pair, qk, c4, eng)

                    def qk_half_thunks(pair, qk, c4):
                        # heavy 8-mm groups split across two filler slots so
                        # PE load stays under ACT's per-slot budget
                        st8 = {}

                        def h1():
                            nm = f"ps{'qk'[qk]}{pair}_{c4}"
                            ps = smp.tile([128, 512], F32, tag="sm", name=nm)
                            st8["ps"] = ps
                            for k in range(KCH // 2):
                                qk_mm(ps, pair, qk, c4, k)

                        def h2():
                            ps = st8["ps"]
                            for k in range(KCH // 2, KCH):
                                qk_mm(ps, pair, qk, c4, k)
                            qk_finish(ps, pair, qk, c4)

                        return [h1, h2]

                    # ---- emission order (priority): get ACT (exp) started ASAP,
                    # then feed PE filler work (pair-1 QK projections, per-chunk
                    # normalization, output projections) into the attention loops
                    # at a rate that keeps ACT (the bottleneck engine) from starving.
                    def placed(norms, fillers, last=None):
                        ex = []
                        if norms:
                            for th in norms:
                                th()
                        ex += list(zip([3, 5, 7, 9, 11, 13, 15, 4, 6], fillers))
                        if last is not None:
                            ex.append((14, last))
                        return ex

                    # V groups feed in one per j-slot (each gates only that j's
                    # delayed PV, never the exp stream); KT chunk c4 is first
                    # read at j=4*c4.
                    if first:
                        # cold start: KT chunks slot in as their c4-quarter
                        # DMAs land (kt chunk c4 is first read at j=4*c4)
                        ex00 = [(3, qk_thunk(0, 1, 1)),
                                (7, qk_thunk(0, 1, 2)),
                                (11, qk_thunk(0, 1, 3)),
                                (13, qk_thunk(0, 0, 1))]
                    else:
                        ex00 = [(4, qk_thunk(0, 1, 3)),
                                (13, qk_thunk(0, 0, 1))]
                    n00 = attn(0, 0, pv_delay=True,
                               pre_av=lambda j: v_group(j),
                               extras=ex00)
                    # pair-1 QK groups spread 2-per-attn across four calls so
                    # no call's filler load starves the exp stream; qt[1]
                    # chunks 0/1 must land by n30 (attn(0,1)/(1,1) read them)
                    n10 = attn(1, 0, pv_delay=True, extras=placed(
                        n00,
                        qk_half_thunks(1, 1, 0) + qk_half_thunks(1, 1, 1),
                        qk_thunk(0, 0, 2)))
                    n20 = attn(2, 0, pv_delay=True, extras=placed(
                        n10,
                        qk_half_thunks(1, 1, 2) + qk_half_thunks(1, 1, 3),
                        qk_thunk(0, 0, 3)))
                    n30 = attn(3, 0, pv_delay=True, extras=placed(
                        n20,
                        qk_half_thunks(1, 0, 0) + qk_half_thunks(1, 0, 1)))
                    n01 = attn(0, 1, pv_delay=True, extras=placed(
                        n30,
                        qk_half_thunks(1, 0, 2) + qk_half_thunks(1, 0, 3)))
                    n11 = attn(1, 1, pv_delay=True, extras=placed(n01, proj_thunks(0)))
                    n21 = attn(2, 1, pv_delay=True, extras=placed(n11, proj_thunks(1)))
                    attn(3, 1, pv_delay=True, extras=placed(n21, proj_thunks(2)))
                    final_tail()

            if loop is None:
                # xT for rep r+1 ships during rep r (big pool holds 2 reps);
                # rep r's tail pre-computes rep r+1's kt0/qt0 so the exp
                # stream restarts immediately at the seam.
                xt_cur = ship_xt(True)
                late_weights_first()
                for rep in range(reps):
                    nxt = ship_xt(False) if rep < reps - 1 else None
                    body(rep == 0, xt=xt_cur, nxt=nxt, pre01_done=(rep > 0))
                    xt_cur = nxt
            else:
                # hardware loop: body emitted once, run `loop` times
                load_late_weights()
                with tc.For_i(0, loop, 1):
                    body(False)

    nc.compile()
    _NC_CACHE[key] = nc
    return nc


def make_in_maps(x, Wq, Wk, Wv, Wo):
    x = np.asarray(x, dtype=np.float32)
    Wq = np.asarray(Wq, dtype=np.float32)
    Wk = np.asarray(Wk, dtype=np.float32)
    Wv = np.asarray(Wv, dtype=np.float32)
    Wo = np.asarray(Wo, dtype=np.float32)
    in_maps = []
    for core in range(NCORES):
        b, g = core // HPC, core % HPC
        cs = slice(GCOLS * g, GCOLS * (g + 1))
        xb = x[b].T.astype(NPBF).reshape(KCH, 128, L).transpose(1, 0, 2)

        def pair_major(w):
            w = w.astype(NPBF)
            return np.stack(
                [
                    w[:, 128 * p : 128 * (p + 1)]
                    .reshape(KCH, 128, 128)
                    .transpose(1, 0, 2)
                    for p in range(2)
                ]
            )

        wq = pair_major(Wq[:, cs])
        wk = pair_major(Wk[:, cs])
        wvp = np.zeros((DIM, HPC, D + 1), dtype=NPBF)
        wvp[:, :, :D] = Wv[:, cs].reshape(DIM, HPC, D)
        wv = wvp.reshape(KCH, 128, HPC, D + 1).transpose(1, 0, 2, 3)
        wo = Wo[cs, :].astype(NPBF).reshape(2, 128, DIM).transpose(1, 0, 2)
        in_maps.append(
            {
                "xT": np.ascontiguousarray(xb),
                "wq": np.ascontiguousarray(wq),
                "wk": np.ascontiguousarray(wk),
                "wv": np.ascontiguousarray(wv),
                "wo": np.ascontiguousarray(wo),
            }
        )
    return in_maps


def combine_outputs(results):
    out = np.zeros((B, L, DIM), dtype=np.float32)
    for core in range(NCORES):
        out[core // HPC] += results[core]["out"].astype(np.float32)
    return out


def kernel(x, Wq, Wk, Wv, Wo):
    nc = _build_nc()
    in_maps = make_in_maps(x, Wq, Wk, Wv, Wo)
    last_err = None
    for _ in range(3):
        try:
            res = run_bass_kernel_spmd(nc, in_maps, core_ids=list(range(NCORES)))
            return combine_outputs(res.results)
        except Exception as e:  # transient NRT/device-unrecoverable states
            last_err = e
    raise last_err

